# revision 2
# baseline (speedup 1.0000x reference)
"""Batched 2048-point DFT on 8 TRN2 NeuronCores — bf16 reversed four-step (v4).

n = 2048 = 16 * 128, m = 128*m1' + m2', k = 16*k1' + k2':
  G[b, k2', m2']    = sum_m1' x[b, 128*m1'+m2'] * W16[m1', k2']     (stage A)
  X[b, 16*k1'+k2'] = sum_m2' B_k2'[m2', k1'] * G[b, k2', m2']       (stage B)
  with B_k2'[m2', k1'] = exp(-2i*pi*m2'*(16*k1'+k2')/2048).

Doing the 16-DFT FIRST lets the host prepare the (bl, m1') partition
interleave for free, so every on-chip access is contiguous:
  - stage A: block-permuted SA stationary -> psum[(k2',bl), (bhi,m2')],
    contiguous psum->sbuf copies into G
  - transposes: contiguous [128,128] chunks of G (single free dim)
  - stage B: stationary B_k2', moving = tg[:, :, k2'*8:+8] (a 2-free-dim
    strided view — PE moving reads are column-wise, stride-free)
  - output dumped transposed [k1', k2', b] with 1KB runs; host un-permutes
All bf16 (tol 2e-2 >> bf16 ~4e-3); few big partition-major DMAs on two
HWDGE queues; PE warmed up with dummy matmuls during DMA startup.
"""

import sys

for _p in ("/opt/trn_rl_repo", "/root/.axon_site/_ro/trn_rl_repo"):
    if _p not in sys.path:
        sys.path.insert(0, _p)

import numpy as np
import ml_dtypes

import concourse.bass as bass
import concourse.mybir as mybir
import concourse.tile as tile
from concourse import bacc
from concourse.bass_utils import run_bass_kernel_spmd
from concourse.masks import make_identity

BATCH = 4096
NFFT = 2048
NCORES = 8
BPC = BATCH // NCORES  # 512
N1 = 128  # m2' / k1'
N2 = 16  # m1' / k2'
NBB = 16  # stage-A column blocks of 512 (4 bhi each)

F32 = mybir.dt.float32
BF16 = mybir.dt.bfloat16
NPBF16 = ml_dtypes.bfloat16

_CACHE = {}

# stage-A x column blocks (of the 8192 (bhi, m2') columns)
XBLOCKS = [(0, 512), (512, 1024), (1024, 2048), (2048, 4096), (4096, 8192)]
# stage-B k2' blocks for output staging/flush; small last block for the tail
KBLOCKS = [(0, 4), (4, 8), (8, 12), (12, 15), (15, 16)]
NWARM = 24


def _build_nc():
    nc = bacc.Bacc("TRN2", target_bir_lowering=False, debug=False)

    # x laid out [p=(bl,m1'), c=(bhi,m2')] by the host
    xre_d = nc.dram_tensor("xre", [N1, 64 * N1], BF16, kind="ExternalInput").ap()
    xim_d = nc.dram_tensor("xim", [N1, 64 * N1], BF16, kind="ExternalInput").ap()
    sa_d = nc.dram_tensor("samat", [N1, 3, N1], BF16, kind="ExternalInput").ap()
    b_d = nc.dram_tensor("bmat", [N1, N2, 3, N1], BF16, kind="ExternalInput").ap()
    # output dumped as [k1', k2', b]; host un-permutes for free
    ore_d = nc.dram_tensor("ore", [N1, N2, BPC], BF16, kind="ExternalOutput").ap()
    oim_d = nc.dram_tensor("oim", [N1, N2, BPC], BF16, kind="ExternalOutput").ap()

    with tile.TileContext(nc) as tc:
        with (
            tc.tile_pool(name="const", bufs=1) as cpool,
            tc.tile_pool(name="x", bufs=1) as xpool,
            tc.tile_pool(name="g", bufs=1) as gpool,
            tc.tile_pool(name="o", bufs=2) as opool,
            tc.tile_pool(name="psA", bufs=3, space="PSUM") as psApool,
            tc.tile_pool(name="pt", bufs=2, space="PSUM") as ptpool,
            tc.tile_pool(name="psB", bufs=2, space="PSUM") as psBpool,
        ):
            sa_t = cpool.tile([128, 3, 128], BF16, tag="samat")
            b_t = cpool.tile([128, N2, 3, 128], BF16, tag="bmat")
            ident = cpool.tile([128, 128], BF16, tag="ident")
            xre_t = xpool.tile([128, 64 * N1], BF16, tag="xre")
            xim_t = xpool.tile([128, 64 * N1], BF16, tag="xim")

            # input DMAs: SA + first x block ASAP (SP queue for xre/consts,
            # Scalar queue for xim), then growing blocks, then B
            c0, c1 = XBLOCKS[0]
            nc.sync.dma_start(sa_t[:], sa_d)
            nc.sync.dma_start(xre_t[:, c0:c1], xre_d[:, c0:c1])
            nc.sync.dma_start(xim_t[:, c0:c1], xim_d[:, c0:c1])
            for c0, c1 in XBLOCKS[1:]:
                nc.sync.dma_start(xre_t[:, c0:c1], xre_d[:, c0:c1])
                nc.sync.dma_start(xim_t[:, c0:c1], xim_d[:, c0:c1])
            nc.sync.dma_start(b_t[:, 0:4], b_d[:, 0:4])
            nc.sync.dma_start(b_t[:, 4:], b_d[:, 4:])
            make_identity(nc, ident[:])

            # G[(k2',bl), (bhi, m2')] and its transpose tg[m2', bhi, (k2',bl)]
            g_re = gpool.tile([128, 64 * N1], BF16, tag="gre")
            g_im = gpool.tile([128, 64 * N1], BF16, tag="gim")
            tg_re = gpool.tile([128, N2, 64, 8], BF16, tag="tgre")
            tg_im = gpool.tile([128, N2, 64, 8], BF16, tag="tgim")
            # copy-destination views ordered like the transpose psum cols
            tgre_v = tg_re[:].rearrange("p k2 bh bl -> p bh k2 bl")
            tgim_v = tg_im[:].rearrange("p k2 bh bl -> p bh k2 bl")

            sare = sa_t[:, 0, :]
            saim = sa_t[:, 1, :]
            saimn = sa_t[:, 2, :]

            # ---- stage A + transposes, per 512-col block (4 bhi) ----
            for bb in range(NBB):
                csl = slice(bb * 512, (bb + 1) * 512)
                ps_re = psApool.tile([128, 512], F32, tag="psA")
                ps_im = psApool.tile([128, 512], F32, tag="psA")
                xr = xre_t[:, csl]
                xi = xim_t[:, csl]
                nc.tensor.matmul(ps_re[:], sare, xr, start=True, stop=False)
                nc.tensor.matmul(ps_im[:], sare, xi, start=True, stop=False)
                nc.tensor.matmul(ps_im[:], saim, xr, start=False, stop=True)
                nc.tensor.matmul(ps_re[:], saimn, xi, start=False, stop=True)
                nc.scalar.copy(g_re[:, csl], ps_re[:])
                nc.vector.tensor_copy(g_im[:, csl], ps_im[:])

                pt_re = ptpool.tile([128, 512], BF16, tag="pt")
                pt_im = ptpool.tile([128, 512], BF16, tag="pt")
                for j in range(4):
                    bh = bb * 4 + j
                    hsl = slice(bh * 128, (bh + 1) * 128)
                    jsl = slice(j * 128, (j + 1) * 128)
                    nc.tensor.transpose(pt_re[:, jsl], g_re[:, hsl], ident[:])
                    nc.tensor.transpose(pt_im[:, jsl], g_im[:, hsl], ident[:])
                bsl = slice(bb * 4, (bb + 1) * 4)
                nc.scalar.copy(tgre_v[:, bsl], pt_re[:])
                nc.vector.tensor_copy(tgim_v[:, bsl], pt_im[:])

            # ---- stage B, per k2' ----
            for k0, k1 in KBLOCKS:
                o_re = opool.tile([128, k1 - k0, BPC], BF16, tag="ore")
                o_im = opool.tile([128, k1 - k0, BPC], BF16, tag="oim")
                for k in range(k0, k1):
                    bre = b_t[:, k, 0, :]
                    bim = b_t[:, k, 1, :]
                    bimn = b_t[:, k, 2, :]
                    tr = tg_re[:, k]  # [128, 64, 8] contiguous moving
                    ti = tg_im[:, k]
                    ps_re = psBpool.tile([128, BPC], F32, tag="psB")
                    ps_im = psBpool.tile([128, BPC], F32, tag="psB")
                    nc.tensor.matmul(ps_re[:], bre, tr, start=True, stop=False)
                    nc.tensor.matmul(ps_im[:], bre, ti, start=True, stop=False)
                    nc.tensor.matmul(ps_im[:], bim, tr, start=False, stop=True)
                    nc.tensor.matmul(ps_re[:], bimn, ti, start=False, stop=True)
                    nc.scalar.copy(o_re[:, k - k0, :], ps_re[:])
                    nc.vector.tensor_copy(o_im[:, k - k0, :], ps_im[:])

                nc.sync.dma_start(ore_d[:, k0:k1], o_re[:])
                nc.sync.dma_start(oim_d[:, k0:k1], o_im[:])

    nc.compile()
    return nc


def _consts():
    m1 = np.arange(N2, dtype=np.float64)
    k2 = np.arange(N2, dtype=np.float64)
    # SA[p=(bl,m1'), v, i=(k2',bl)] block-permuted W16
    ph16 = -2.0 * np.pi * np.outer(m1, k2) / N2
    w16re = np.cos(ph16).astype(np.float32)
    w16im = np.sin(ph16).astype(np.float32)
    sa = np.zeros((N1, 3, N1), np.float32)
    for bl in range(8):
        sa[bl * 16 : (bl + 1) * 16, 0, bl::8] = w16re
        sa[bl * 16 : (bl + 1) * 16, 1, bl::8] = w16im
        sa[bl * 16 : (bl + 1) * 16, 2, bl::8] = -w16im
    # B[m2', k2', v, k1']
    m2v = np.arange(N1, dtype=np.float64)[:, None, None]
    k2v = np.arange(N2, dtype=np.float64)[None, :, None]
    k1v = np.arange(N1, dtype=np.float64)[None, None, :]
    ph = -2.0 * np.pi * m2v * (16.0 * k1v + k2v) / NFFT
    b_arr = np.empty((N1, N2, 3, N1), np.float32)
    b_arr[:, :, 0, :] = np.cos(ph)
    b_arr[:, :, 1, :] = np.sin(ph)
    b_arr[:, :, 2, :] = -b_arr[:, :, 1, :]
    return sa.astype(NPBF16), b_arr.astype(NPBF16)


def run(signal_re, signal_im, trace=False, tmpdir=None):
    if "nc" not in _CACHE:
        _CACHE["nc"] = _build_nc()
        _CACHE["c"] = _consts()
    nc = _CACHE["nc"]
    samat, bmat = _CACHE["c"]

    sre = np.asarray(signal_re, dtype=np.float32)
    sim = np.asarray(signal_im, dtype=np.float32)

    in_maps = []
    for c in range(NCORES):
        bsl = slice(c * BPC, (c + 1) * BPC)
        # xA[p=(bl,m1'), (bhi,m2')] from x[b=(bhi,bl), m=(m1',m2')]
        xre = np.ascontiguousarray(
            sre[bsl].reshape(64, 8, N2, N1).transpose(1, 2, 0, 3).reshape(N1, 64 * N1)
        ).astype(NPBF16)
        xim = np.ascontiguousarray(
            sim[bsl].reshape(64, 8, N2, N1).transpose(1, 2, 0, 3).reshape(N1, 64 * N1)
        ).astype(NPBF16)
        in_maps.append({"xre": xre, "xim": xim, "samat": samat, "bmat": bmat})

    last_exc = None
    for attempt in range(3):
        try:
            br = run_bass_kernel_spmd(
                nc, in_maps, list(range(NCORES)), trace=trace, tmpdir=tmpdir
            )
            break
        except Exception as e:
            last_exc = e
            import time

            time.sleep(2.0)
    else:
        raise last_exc

    out_re = np.empty((BATCH, NFFT), np.float32)
    out_im = np.empty((BATCH, NFFT), np.float32)
    for c in range(NCORES):
        bsl = slice(c * BPC, (c + 1) * BPC)
        for name, dst in (("ore", out_re), ("oim", out_im)):
            o = np.asarray(br.results[c][name]).astype(np.float32)
            # O[k1', k2', b] -> X[b, k1'*16 + k2']
            dst[bsl, :] = o.transpose(2, 0, 1).reshape(BPC, NFFT)
    return (out_re, out_im), br


def kernel(signal_re, signal_im):
    return run(signal_re, signal_im)[0]
